# revision 1
# baseline (speedup 1.0000x reference)
"""Trainium2 Bass kernel for the Involution module (B=4, C=64, H=W=128, K=7, G=4).

Algorithm per core (8-way data parallel: core = (batch, h-half)):
  - layout: partition p = channel c + 64*hb, hb = which 32-row half-of-half;
    free dim = zero-padded 38x134 pixel slab (halo rows included).
  - 1x1 kernel-generating conv as matmuls (K=64 contract over channels) into
    PSUM, fused BN+SiLU on ScalarE (per-partition scale/bias) -> bf16 SBUF.
  - 16x channel-replication of the per-pixel kernels with tiny-K matmuls
    (one-hot selection lhsT), ScalarE copy PSUM->SBUF bf16.
  - involution MAC: 49 shifted tensor_tensor mult/add on VectorE (bf16 2x
    mode; an element-shifted copy of x keeps all windows 4B-aligned).
"""

import os

os.environ.setdefault("JAX_PLATFORMS", "cpu")

import numpy as np
import ml_dtypes

import concourse.bacc as bacc
import concourse.tile as tile
import concourse.mybir as mybir
from concourse.bass_utils import run_bass_kernel_spmd

# Problem constants (hardcoded per harness contract).
B, C, H, W = 4, 64, 128, 128
K, G, GC = 7, 4, 16
KK = K * K
KO = KK * G  # 196
PAD = 3
BN_EPS = 1e-5

HB_ROWS = 32          # rows per half-of-half (per partition group)
SLAB_R = HB_ROWS + 6  # 38 padded rows per hb slab
SLAB_W = W + 6        # 134 padded cols
SLAB_F = SLAB_R * SLAB_W
RPC = 4               # output rows per pixel chunk (512 px)
NCHUNK = HB_ROWS // RPC  # 8 chunks
CHW = RPC * W         # 512 free elements per chunk

# M-chunking of the 196 KO channels: chunk1 = (g, k<32) -> 128 rows,
# chunk2 = (g, 32+kk) kk<17 -> 68 rows.
M1, M2 = 128, 68
K1 = 32  # k values in chunk1 per group

USE_BF16 = True


def _dt():
    return mybir.dt.bfloat16 if USE_BF16 else mybir.dt.float32


def _npdt():
    return ml_dtypes.bfloat16 if USE_BF16 else np.float32


def build_bass():
    nc = bacc.Bacc(
        "TRN2",
        target_bir_lowering=False,
        debug=False,
        enable_asserts=False,
        num_devices=8,
    )
    DT = _dt()
    f32 = mybir.dt.float32

    xq_d = nc.dram_tensor("xq", [128, SLAB_F], DT, kind="ExternalInput").ap()
    ws1_d = nc.dram_tensor("ws1", [128, M1], DT, kind="ExternalInput").ap()
    ws2_d = nc.dram_tensor("ws2", [128, M2], DT, kind="ExternalInput").ap()
    e1_d = nc.dram_tensor("e1", [M1, K1 * 64], DT, kind="ExternalInput").ap()
    e2_d = nc.dram_tensor("e2", [M2, (KK - K1) * 64], DT, kind="ExternalInput").ap()
    sc1_d = nc.dram_tensor("sc1", [M1, 1], f32, kind="ExternalInput").ap()
    sh1_d = nc.dram_tensor("sh1", [M1, 1], f32, kind="ExternalInput").ap()
    sc2_d = nc.dram_tensor("sc2", [M2, 1], f32, kind="ExternalInput").ap()
    sh2_d = nc.dram_tensor("sh2", [M2, 1], f32, kind="ExternalInput").ap()
    out_d = nc.dram_tensor("out", [128, HB_ROWS * W], f32, kind="ExternalOutput").ap()

    with tile.TileContext(nc) as tc:
        build_kernel(
            tc, xq_d, ws1_d, ws2_d, e1_d, e2_d, sc1_d, sh1_d, sc2_d, sh2_d, out_d
        )
    nc.compile()
    return nc


def build_kernel(tc, xq_d, ws1_d, ws2_d, e1_d, e2_d, sc1_d, sh1_d, sc2_d, sh2_d, out_d):
    from contextlib import ExitStack

    nc = tc.nc
    DT = _dt()
    f32 = mybir.dt.float32

    ctx = ExitStack()
    consts = ctx.enter_context(tc.tile_pool(name="consts", bufs=1))
    wwpool = ctx.enter_context(tc.tile_pool(name="ww", bufs=2))
    wxpool = ctx.enter_context(tc.tile_pool(name="wx", bufs=4))
    tmppool = ctx.enter_context(tc.tile_pool(name="tmp", bufs=3))
    accpool = ctx.enter_context(tc.tile_pool(name="acc", bufs=2))
    outpool = ctx.enter_context(tc.tile_pool(name="outf", bufs=2))
    zpool = ctx.enter_context(tc.tile_pool(name="z", bufs=1, space="PSUM"))
    wepool = ctx.enter_context(tc.tile_pool(name="wexp", bufs=4, space="PSUM"))

    xq = consts.tile([128, SLAB_F], DT)
    nc.sync.dma_start(out=xq, in_=xq_d)
    ws1 = consts.tile([128, M1], DT)
    nc.sync.dma_start(out=ws1, in_=ws1_d)
    ws2 = consts.tile([128, M2], DT)
    nc.sync.dma_start(out=ws2, in_=ws2_d)
    e1 = consts.tile([M1, K1, 64], DT)
    nc.sync.dma_start(out=e1, in_=e1_d.rearrange("p (k c) -> p k c", k=K1))
    e2 = consts.tile([M2, KK - K1, 64], DT)
    nc.sync.dma_start(out=e2, in_=e2_d.rearrange("p (k c) -> p k c", k=KK - K1))
    sc1 = consts.tile([M1, 1], f32)
    nc.sync.dma_start(out=sc1, in_=sc1_d)
    sh1 = consts.tile([M1, 1], f32)
    nc.sync.dma_start(out=sh1, in_=sh1_d)
    sc2 = consts.tile([M2, 1], f32)
    nc.sync.dma_start(out=sc2, in_=sc2_d)
    sh2 = consts.tile([M2, 1], f32)
    nc.sync.dma_start(out=sh2, in_=sh2_d)

    # Element-shifted copy of the slab so odd-dw windows stay 4B-aligned
    # (keeps the DVE in bf16 2x mode).
    if USE_BF16:
        xqo = consts.tile([128, SLAB_F], DT)
        nc.vector.tensor_copy(xqo[:, 0 : SLAB_F - 2], xq[:, 1 : SLAB_F - 1])
    else:
        xqo = None

    silu = mybir.ActivationFunctionType.Silu
    xq3 = xq.rearrange("p (r w) -> p r w", w=SLAB_W)
    xqo3 = xqo.rearrange("p (r w) -> p r w", w=SLAB_W) if xqo is not None else None

    for j in range(NCHUNK):
        # ---- 1x1 conv for this chunk's pixels (both halves) ----
        # interior window: slab rows 4j+3..4j+7, cols 3..130
        ww = {}
        for hb in range(2):
            p0 = 64 * hb
            rhs = xq3[p0 : p0 + 64, RPC * j + PAD : RPC * j + PAD + RPC, PAD : PAD + W]
            z1 = zpool.tile([128, CHW], f32, tag=f"z1{hb}")
            nc.tensor.matmul(
                z1,
                ws1[p0 : p0 + 64, :],
                rhs,
                start=True,
                stop=True,
            )
            z2 = zpool.tile([M2, CHW], f32, tag=f"z2{hb}")
            nc.tensor.matmul(
                z2,
                ws2[p0 : p0 + 64, :],
                rhs,
                start=True,
                stop=True,
            )
            # BN + SiLU on ScalarE, per-partition scale/bias -> bf16 SBUF
            w1 = wwpool.tile([128, CHW], DT, tag=f"ww1{hb}")
            nc.scalar.activation(w1, z1, silu, bias=sh1, scale=sc1)
            w2 = wwpool.tile([M2, CHW], DT, tag=f"ww2{hb}")
            nc.scalar.activation(w2, z2, silu, bias=sh2[0:M2], scale=sc2[0:M2])
            ww[hb] = (w1, w2)

        # ---- involution MAC over the 49 kernel positions ----
        # two bf16 accumulators (even/odd k) + fp32 combine: halves the
        # sequential-rounding walk of the accumulation
        acc = accpool.tile([128, CHW], DT, tag="acc")
        acc2 = accpool.tile([128, CHW], DT, tag="acc2")
        outf = outpool.tile([128, CHW], f32, tag="outf")
        for k in range(KK):
            dh, dw = k // K, k % K
            # expanded per-pixel kernel values: wexp[c + 64*hb, pix]
            wexp = wepool.tile([128, CHW], f32, tag="wexp")
            lhsT = e1[:, k, :] if k < K1 else e2[:, k - K1, :]
            for hb in range(2):
                w1g, w2g = ww[hb]
                src = w1g if k < K1 else w2g
                nc.tensor.matmul(
                    wexp[64 * hb : 64 * hb + 64, :],
                    lhsT,
                    src,
                    start=True,
                    stop=True,
                )
            wx = wxpool.tile([128, CHW], DT, tag="wx")
            nc.scalar.copy(wx, wexp)

            # shifted x window for this (dh, dw)
            r0 = RPC * j + dh
            if USE_BF16 and (dw % 2 == 1):
                xwin = xqo3[:, r0 : r0 + RPC, dw - 1 : dw - 1 + W]
            else:
                xwin = xq3[:, r0 : r0 + RPC, dw : dw + W]

            a = acc if k % 2 == 0 else acc2
            if k < 2:
                nc.vector.tensor_mul(a, xwin, wx)
            else:
                t = tmppool.tile([128, CHW], DT, tag="tmp")
                nc.vector.tensor_mul(t, xwin, wx)
                nc.vector.tensor_add(a, a, t)
        nc.vector.tensor_add(outf, acc, acc2)

        nc.sync.dma_start(out=out_d[:, j * CHW : (j + 1) * CHW], in_=outf)
    ctx.close()


def prep_inputs(x, conv_w, bn_gamma, bn_beta, bn_mean, bn_var):
    """Host-side prep: per-core padded slabs + shared weight tables."""
    npdt = _npdt()
    scale = (bn_gamma / np.sqrt(bn_var + BN_EPS)).astype(np.float32)
    shift = (bn_beta - bn_mean * scale).astype(np.float32)

    # KO index maps for the two M-chunks
    m1 = np.arange(M1)
    ko1 = (m1 // K1) * KK + (m1 % K1)
    m2 = np.arange(M2)
    ko2 = (m2 // 17) * KK + K1 + (m2 % 17)

    ws1 = np.zeros((128, M1), npdt)
    ws1[0:64] = conv_w[ko1].T.astype(npdt)
    ws1[64:128] = ws1[0:64]
    ws2 = np.zeros((128, M2), npdt)
    ws2[0:64] = conv_w[ko2].T.astype(npdt)
    ws2[64:128] = ws2[0:64]

    e1 = np.zeros((M1, K1, 64), npdt)
    for g in range(G):
        for k in range(K1):
            e1[g * K1 + k, k, g * GC : (g + 1) * GC] = 1.0
    e2 = np.zeros((M2, KK - K1, 64), npdt)
    for g in range(G):
        for kk in range(KK - K1):
            e2[g * 17 + kk, kk, g * GC : (g + 1) * GC] = 1.0

    sc1 = scale[ko1].reshape(M1, 1)
    sh1 = shift[ko1].reshape(M1, 1)
    sc2 = scale[ko2].reshape(M2, 1)
    sh2 = shift[ko2].reshape(M2, 1)

    xp = np.zeros((B, C, H + 2 * PAD, W + 2 * PAD), npdt)
    xp[:, :, PAD : PAD + H, PAD : PAD + W] = x.astype(npdt)

    in_maps = []
    for core in range(8):
        b, half = core // 2, core % 2
        h0 = 64 * half
        xq = np.zeros((128, SLAB_F), npdt)
        for hb in range(2):
            r0 = h0 + HB_ROWS * hb  # first output row of this hb (unpadded idx)
            slab = xp[b, :, r0 : r0 + SLAB_R, :]  # [64, 38, 134] (padded idx r0..)
            xq[64 * hb : 64 * hb + 64] = slab.reshape(C, SLAB_F)
        in_maps.append(
            {
                "xq": xq,
                "ws1": ws1,
                "ws2": ws2,
                "e1": e1.reshape(M1, K1 * 64),
                "e2": e2.reshape(M2, (KK - K1) * 64),
                "sc1": sc1,
                "sh1": sh1,
                "sc2": sc2,
                "sh2": sh2,
            }
        )
    return in_maps


def assemble_output(results):
    out = np.zeros((B, C, H, W), np.float32)
    for core in range(8):
        b, half = core // 2, core % 2
        h0 = 64 * half
        oc = results[core]["out"].reshape(128, HB_ROWS, W)
        for hb in range(2):
            out[b, :, h0 + HB_ROWS * hb : h0 + HB_ROWS * (hb + 1), :] = oc[
                64 * hb : 64 * hb + 64
            ]
    return out


def kernel(x, conv_w, bn_gamma, bn_beta, bn_mean, bn_var):
    x = np.asarray(x, np.float32)
    conv_w = np.asarray(conv_w, np.float32)
    in_maps = prep_inputs(
        x,
        conv_w,
        np.asarray(bn_gamma, np.float32),
        np.asarray(bn_beta, np.float32),
        np.asarray(bn_mean, np.float32),
        np.asarray(bn_var, np.float32),
    )
    nc = build_bass()
    res = run_bass_kernel_spmd(nc, in_maps, core_ids=list(range(8)))
    return assemble_output(res.results)


if __name__ == "__main__":
    rng = np.random.default_rng(0)
    ins = {
        "x": rng.standard_normal((B, C, H, W), np.float32),
        "conv_w": rng.standard_normal((KO, C), np.float32) / 8.0,
        "bn_gamma": rng.uniform(0.5, 1.5, KO).astype(np.float32),
        "bn_beta": rng.standard_normal(KO).astype(np.float32) * 0.1,
        "bn_mean": rng.standard_normal(KO).astype(np.float32) * 0.1,
        "bn_var": rng.uniform(0.5, 1.5, KO).astype(np.float32),
    }
    out = kernel(**ins)
    print("kernel output", out.shape, out.dtype, np.abs(out).sum())



# revision 2
# speedup vs baseline: 1.5054x; 1.5054x over previous
"""Trainium2 Bass kernel for the Involution module (B=4, C=64, H=W=128, K=7, G=4).

v2 design — pixel-partition layout, no weight-expansion passes:
  - 8-way sharding: core = (batch b, W-half wh): all 128 H-rows, 64 cols.
  - SBUF partitions = 128 H-rows for all main tensors.
  - Transposed 1x1 conv: per col pl, matmul lhsT = x-block [65, 128r]
    (64 channels + ones row for the BN bias), rhs = Wt [65, 196] (BN scale
    folded, column order (k, g)) -> psum [128r, 196] -> ScalarE SiLU ->
    w' bf16 [128, (pl, k, g)].  The per-pixel kernels are born in
    pixel-partition layout: no transposes, no 16x channel expansion.
  - 49 DVE tensor_mul ops: q_k[r, (pl, j, g)] = w'[r, (pl, k, g)]
    (j broadcast via stride-0 free dim) * xt_dh[r, (pl+dw, j, g)]
    where xt_dh is a host-staged row-shifted copy of x (7 copies, one per
    dh; zero rows at the H boundary).  Innermost free dim is the group
    channel g (stride 1, 4 elems), so every dw window stays 4B-aligned:
    all muls run in DVE 2x bf16 mode.
  - PE accumulates the 49 products into 8 PSUM banks via identity
    matmuls (fp32 accumulation), DMA straight from PSUM to HBM.
"""

import os

os.environ.setdefault("JAX_PLATFORMS", "cpu")

import numpy as np
import ml_dtypes

import concourse.bacc as bacc
import concourse.tile as tile
import concourse.mybir as mybir
from concourse.bass_utils import run_bass_kernel_spmd

# Problem constants (hardcoded per harness contract).
B, C, H, W = 4, 64, 128, 128
K, G, GC = 7, 4, 16
KK = K * K
KO = KK * G  # 196
PAD = 3
BN_EPS = 1e-5

NPL = 64          # output cols per core (W-half)
XT_COLS = NPL + 2 * PAD  # 70 stored cols (halo included)
XT_F = XT_COLS * C       # 4480 free elems per dh-copy
QF = NPL * GC * G        # 4096 free elems of q / out
WQF = NPL * KO           # 12544 free elems of w'

USE_BF16 = True


def _dt():
    return mybir.dt.bfloat16 if USE_BF16 else mybir.dt.float32


def _npdt():
    return ml_dtypes.bfloat16 if USE_BF16 else np.float32


def build_bass():
    nc = bacc.Bacc(
        "TRN2",
        target_bir_lowering=False,
        debug=False,
        enable_asserts=False,
        num_devices=8,
    )
    DT = _dt()
    f32 = mybir.dt.float32

    xc_d = nc.dram_tensor("xc", [C + 1, NPL * H], DT, kind="ExternalInput").ap()
    wt_d = nc.dram_tensor("wt", [C + 1, KO], DT, kind="ExternalInput").ap()
    id_d = nc.dram_tensor("ident", [128, 128], DT, kind="ExternalInput").ap()
    xt_d = [
        nc.dram_tensor(f"xt{dh}", [128, XT_F], DT, kind="ExternalInput").ap()
        for dh in range(K)
    ]
    out_d = nc.dram_tensor("out", [128, QF], f32, kind="ExternalOutput").ap()

    with tile.TileContext(nc) as tc:
        build_kernel(tc, xc_d, wt_d, id_d, xt_d, out_d)
    nc.compile()
    return nc


def build_kernel(tc, xc_d, wt_d, id_d, xt_d, out_d):
    from contextlib import ExitStack

    nc = tc.nc
    DT = _dt()
    f32 = mybir.dt.float32
    silu = mybir.ActivationFunctionType.Silu

    ctx = ExitStack()
    consts = ctx.enter_context(tc.tile_pool(name="consts", bufs=1))
    qpool = ctx.enter_context(tc.tile_pool(name="q", bufs=3))

    xc = consts.tile([C + 1, NPL * H], DT)
    nc.sync.dma_start(out=xc, in_=xc_d)
    wt = consts.tile([C + 1, KO], DT)
    nc.sync.dma_start(out=wt, in_=wt_d)
    ident = consts.tile([128, 128], DT)
    nc.sync.dma_start(out=ident, in_=id_d)
    xt = []
    for dh in range(K):
        t = consts.tile([128, XT_F], DT)
        nc.sync.dma_start(out=t, in_=xt_d[dh])
        xt.append(t)
    wq = consts.tile([128, WQF], DT)

    # ---- transposed 1x1 conv + BN + SiLU -> w'[r, (pl, k, g)] ----
    xc3 = xc.rearrange("p (pl r) -> p pl r", pl=NPL)
    with tc.tile_pool(name="zp", bufs=8, space="PSUM") as zpool:
        for pl in range(NPL):
            z = zpool.tile([128, KO], f32, tag="z")
            nc.tensor.matmul(z, xc3[:, pl, :], wt, start=True, stop=True)
            nc.scalar.activation(wq[:, pl * KO : (pl + 1) * KO], z, silu)

    # ---- involution MAC: 49 broadcast muls + PE accumulation ----
    wq4 = wq.rearrange("p (pl k g) -> p pl k g", k=KK, g=G)
    NCC = QF // 512  # 8 psum accumulator banks
    with tc.tile_pool(name="acc", bufs=1, space="PSUM") as accpool:
        accs = [
            accpool.tile([128, 512], f32, tag=f"acc{cc}", name=f"acc{cc}")
            for cc in range(NCC)
        ]
        for k in range(KK):
            dh, dw = divmod(k, K)
            q = qpool.tile([128, QF], DT, tag="q")
            qv = q.rearrange("p (pl j g) -> p pl j g", j=GC, g=G)
            wv = wq4[:, :, k : k + 1, :].broadcast_to([128, NPL, GC, G])
            xv = xt[dh][:, dw * C : dw * C + QF].rearrange(
                "p (pl j g) -> p pl j g", j=GC, g=G
            )
            nc.vector.tensor_mul(qv, wv, xv)
            for cc in range(NCC):
                nc.tensor.matmul(
                    accs[cc],
                    ident,
                    q[:, cc * 512 : (cc + 1) * 512],
                    start=(k == 0),
                    stop=(k == KK - 1),
                )
        outf = consts.tile([128, QF], f32)
        for cc in range(NCC):
            sl = slice(cc * 512, (cc + 1) * 512)
            nc.scalar.copy(outf[:, sl], accs[cc])
            nc.sync.dma_start(out=out_d[:, sl], in_=outf[:, sl])
    ctx.close()


def prep_inputs(x, conv_w, bn_gamma, bn_beta, bn_mean, bn_var):
    """Host-side prep: per-core staged tensors (bf16)."""
    npdt = _npdt()
    scale = (bn_gamma / np.sqrt(bn_var + BN_EPS)).astype(np.float32)
    shift = (bn_beta - bn_mean * scale).astype(np.float32)

    # Wt[c, k*4+g] = conv_w[g*49+k, c] * scale ; row 64 = shift (bias row).
    o = (np.arange(G)[None, :] * KK + np.arange(KK)[:, None]).reshape(-1)  # (k,g)
    wt = np.zeros((C + 1, KO), npdt)
    wt[0:C] = (conv_w[o].T * scale[o][None, :]).astype(npdt)
    wt[C] = shift[o].astype(npdt)

    ident = np.eye(128, dtype=npdt)

    in_maps = []
    for core in range(8):
        b, wh = divmod(core, 2)
        c0 = NPL * wh
        # xc[c, pl, r] = x[b, c, r, c0+pl]; ones row.
        xc = np.ones((C + 1, NPL, H), npdt)
        xc[0:C] = x[b, :, :, c0 : c0 + NPL].transpose(0, 2, 1).astype(npdt)

        # xt[dh][r, t, c'] = x[b, perm[c'], r+dh-3, c0-3+t], zeros out of
        # range.  Slot c' = j*4+g holds channel g*16+j (the q/w' free axis
        # is (j, g) with g innermost so dw windows stay 4B-aligned).
        jj, gg = np.meshgrid(np.arange(GC), np.arange(G), indexing="ij")
        perm = (gg * GC + jj).reshape(-1)
        xpadW = np.zeros((C, H, W + 2 * PAD), npdt)
        xpadW[:, :, PAD : PAD + W] = x[b, perm].astype(npdt)
        m = {
            "xc": xc.reshape(C + 1, NPL * H),
            "wt": wt,
            "ident": ident,
        }
        for dh in range(K):
            t = np.zeros((H, XT_COLS, C), npdt)
            rlo, rhi = max(0, PAD - dh), min(H, H + PAD - dh)
            t[rlo:rhi] = xpadW[:, rlo + dh - PAD : rhi + dh - PAD, c0 : c0 + XT_COLS].transpose(1, 2, 0)
            m[f"xt{dh}"] = t.reshape(H, XT_F)
        in_maps.append(m)
    return in_maps


def assemble_output(results):
    out = np.zeros((B, C, H, W), np.float32)
    for core in range(8):
        b, wh = divmod(core, 2)
        c0 = NPL * wh
        oc = results[core]["out"].reshape(H, NPL, GC, G)  # [r, pl, j, g]
        out[b, :, :, c0 : c0 + NPL] = oc.transpose(3, 2, 0, 1).reshape(C, H, NPL)
    return out


def kernel(x, conv_w, bn_gamma, bn_beta, bn_mean, bn_var):
    x = np.asarray(x, np.float32)
    conv_w = np.asarray(conv_w, np.float32)
    in_maps = prep_inputs(
        x,
        conv_w,
        np.asarray(bn_gamma, np.float32),
        np.asarray(bn_beta, np.float32),
        np.asarray(bn_mean, np.float32),
        np.asarray(bn_var, np.float32),
    )
    nc = build_bass()
    res = run_bass_kernel_spmd(nc, in_maps, core_ids=list(range(8)))
    return assemble_output(res.results)


if __name__ == "__main__":
    rng = np.random.default_rng(0)
    ins = {
        "x": rng.standard_normal((B, C, H, W), np.float32),
        "conv_w": rng.standard_normal((KO, C), np.float32) / 8.0,
        "bn_gamma": rng.uniform(0.5, 1.5, KO).astype(np.float32),
        "bn_beta": rng.standard_normal(KO).astype(np.float32) * 0.1,
        "bn_mean": rng.standard_normal(KO).astype(np.float32) * 0.1,
        "bn_var": rng.uniform(0.5, 1.5, KO).astype(np.float32),
    }
    out = kernel(**ins)
    print("kernel output", out.shape, out.dtype, np.abs(out).sum())
